# revision 15
# baseline (speedup 1.0000x reference)
import sys
for _p in ("/opt/trn_rl_repo",):
    if _p not in sys.path:
        sys.path.insert(0, _p)
"""GAT 2-layer kernel for TRN2, 8-core dst-sharded — v2 "lane layout".

This environment has a large fixed per-instruction dispatch cost, so the
design minimizes instruction count:
  * nodes are relabeled (host) so each dst window's 128 nodes sit on 128
    SBUF partitions ("lanes"); a window's edges live at [lane, j] with
    j < K_w (degree-balanced relabeling keeps K_w ~ mean degree)
  * per-edge softmax weights and aggregation are pure DVE ops: al_dst add
    is a free-dim broadcast, the neighbor sum is one strided-AP reduce —
    no indicator matmuls, no per-tile transposes
  * int16 gather indices cannot span 50k table rows, so each edge block is
    fetched in an even/odd row-parity pass (the other parity reads a zero
    row) and the two passes are summed
Tables are bf16; weights/x are cast host-side.
"""

import numpy as np
import ml_dtypes
from contextlib import ExitStack

import concourse.bass as bass
import concourse.bacc as bacc
import concourse.mybir as mybir
import concourse.tile as tile

dt = mybir.dt
F32 = dt.float32
BF16 = dt.bfloat16
AL = mybir.AluOpType
ACT = mybir.ActivationFunctionType
BF = ml_dtypes.bfloat16

NEG_SLOPE = 0.2
_CORES = 8
WIN = 128
GCAP = 896
MAX_TILES = 72


def wrap_idx(iv):
    """[n*16] int -> [128, n] int16 gather layout (t -> (t%16, t//16))."""
    iv = np.asarray(iv, np.int64)
    assert len(iv) % 16 == 0
    cols = len(iv) // 16
    a = iv.reshape(cols, 16).T.astype(np.int16)
    return np.tile(a, (8, 1))


def make_cfg(N, E, F_in, H, D, OUT, cores):
    own = (N + cores - 1) // cores
    nw = (own + WIN - 1) // WIN
    ownp = nw * WIN
    cfg = dict(
        N=N, E=E, F_IN=F_in, H=H, D=D, OUT=OUT, CORES=cores, OWN=own,
        NW=nw, OWNP=ownp,
        ROW0=384, ROW1=256,            # bf16 elems per table row
        C0=F_in + 2 * H,               # wcat0 cols (h | al_s | al_d)
        C1=OUT + 2 + OUT,              # wcat1 cols (hW1 | als | ald | res)
        NROWS=4 + cores * ownp,        # [Ze, Zo, Pe, x] + nodes
    )
    assert cfg["NROWS"] % 2 == 0
    cfg["VROWS"] = cfg["NROWS"] // 2
    assert cfg["VROWS"] - 2 <= 32767
    return cfg


def prepare(x, edge_index, weights, cfg):
    N, E, C, OWN, OWNP, NW = (cfg[k] for k in
                              ("N", "E", "CORES", "OWN", "OWNP", "NW"))
    H, D, F_IN, OUT = cfg["H"], cfg["D"], cfg["F_IN"], cfg["OUT"]

    src = np.concatenate([np.asarray(edge_index[0]),
                          np.arange(N)]).astype(np.int64)
    dst = np.concatenate([np.asarray(edge_index[1]),
                          np.arange(N)]).astype(np.int64)

    # degree-balanced relabeling: node order[i] -> (core i%C, pos i//C)
    deg = np.bincount(dst, minlength=N)
    order = np.argsort(-deg, kind="stable")
    core_of = np.empty(N, np.int64)
    pos_of = np.empty(N, np.int64)
    core_of[order] = np.arange(N) % C
    pos_of[order] = np.arange(N) // C

    # per-window max lane count (shared across cores)
    cnt = np.zeros((C, OWNP), np.int64)
    np.add.at(cnt, (core_of[dst], pos_of[dst]), 1)
    Kw = cnt.reshape(C, NW, WIN).max(axis=(0, 2))

    # batches of windows with uniform K, capped at MAX_TILES j-tiles
    batches = []
    w = 0
    while w < NW:
        K = max(int(Kw[w]), 1)
        assert K <= MAX_TILES, f"window degree {K} exceeds MAX_TILES"
        nw_b = 1
        while w + nw_b < NW:
            K2 = max(K, int(Kw[w + nw_b]))
            if (nw_b + 1) * K2 > MAX_TILES:
                break
            K = K2
            nw_b += 1
        batches.append((w, nw_b, K))
        w += nw_b

    slot0_w = np.zeros(NW, np.int64)
    base = 0
    for (w0, nw_b, K) in batches:
        for wl in range(nw_b):
            slot0_w[w0 + wl] = base + wl * K * WIN
        base += nw_b * K * WIN
    totslot = base
    assert totslot % 16 == 0

    # per-edge slot assignment: j = rank within (core,pos) group
    ec, ep = core_of[dst], pos_of[dst]
    order_e = np.lexsort((src, ep, ec))
    src_s, ec_s, ep_s = src[order_e], ec[order_e], ep[order_e]
    grp = ec_s * OWNP + ep_s
    starts = np.searchsorted(grp, np.arange(C * OWNP))
    j_of = np.arange(len(grp)) - starts[grp]
    wd = ep_s // WIN
    lane = ep_s % WIN
    slot = slot0_w[wd] + j_of * WIN + lane
    srow = 4 + core_of[src_s] * OWNP + pos_of[src_s]   # table row of source
    even = (srow % 2 == 0)

    in_maps = []
    for c in range(C):
        m = ec_s == c
        rowE = np.full(totslot, 1, np.int64)   # default: PAD row (view idx 1)
        rowO = np.zeros(totslot, np.int64)     # default: zero row
        sl, sr, ev = slot[m], srow[m], even[m]
        rowE[sl[ev]] = sr[ev] // 2
        rowO[sl[ev]] = 0
        rowE[sl[~ev]] = 0
        rowO[sl[~ev]] = (sr[~ev] - 1) // 2
        nodes = np.full(OWNP, -1, np.int64)
        mine = core_of == c
        nodes[pos_of[mine]] = np.where(mine)[0]
        xT = np.zeros((F_IN, OWNP), BF)
        valid = nodes >= 0
        xT[:, valid] = np.asarray(x, np.float32)[nodes[valid]].T.astype(BF)
        in_maps.append(dict(
            xT=xT,
            idxE=wrap_idx(rowE), idxO=wrap_idx(rowO),
        ))

    # shared constants
    W0 = weights["W0"]; a_s0 = weights["a_src0"]; a_d0 = weights["a_dst0"]
    Wr0 = weights["Wr0"]; W1 = weights["W1"]; a_s1 = weights["a_src1"]
    a_d1 = weights["a_dst1"]; Wr1 = weights["Wr1"]
    blk0s = np.zeros((H * D, H), np.float32)
    blk0d = np.zeros((H * D, H), np.float32)
    for k in range(H):
        blk0s[k * D:(k + 1) * D, k] = a_s0[k]
        blk0d[k * D:(k + 1) * D, k] = a_d0[k]
    wcat0 = np.concatenate([W0, W0 @ blk0s, W0 @ blk0d], axis=1).astype(BF)
    wcat1 = np.concatenate([W1, W1 @ a_s1.reshape(-1, 1),
                            W1 @ a_d1.reshape(-1, 1), Wr1], axis=1).astype(BF)
    bias0 = (weights["b0"] + weights["br0"]).astype(np.float32)   # [256]
    bias_row1 = np.zeros((1, cfg["C1"]), BF)
    bias_row1[0, OUT + 2:] = (weights["b1"] + weights["br1"]).astype(BF)
    consts = dict(
        wcat0=wcat0, wr0=np.asarray(Wr0, np.float32).astype(BF),
        wcat1=wcat1, bias_row1=bias_row1,
        bias0=np.ascontiguousarray(bias0.reshape(2, 128).T),  # [128, 2]
    )
    for m in in_maps:
        m.update(consts)
    meta = dict(batches=batches, totslot=totslot,
                core_of=core_of, pos_of=pos_of,
                bias1_nz=bool(np.any(bias_row1)))
    return in_maps, meta


def build(cfg, meta, repeat=1, abl=()):
    NOCOLL = "nocoll" in abl
    NOGATHER = "nogather" in abl
    N, C, OWN, OWNP, NW = (cfg[k] for k in
                           ("N", "CORES", "OWN", "OWNP", "NW"))
    F_IN, H, D, OUT = cfg["F_IN"], cfg["H"], cfg["D"], cfg["OUT"]
    ROW0, ROW1, C0, C1 = cfg["ROW0"], cfg["ROW1"], cfg["C0"], cfg["C1"]
    VROWS = cfg["VROWS"]
    batches = meta["batches"]
    TOTSLOT = meta["totslot"]
    BIAS1 = meta.get("bias1_nz", True)
    KCH = (F_IN + 127) // 128   # 2

    nc = bacc.Bacc("TRN2", target_bir_lowering=False, debug=False,
                   num_devices=C)

    # hoisted gpsimd registers for gather valid-counts (one RegisterMove per
    # distinct value instead of one per dma_gather call)
    _nreg_cache = {}

    def nreg(v):
        if v not in _nreg_cache:
            _nreg_cache[v] = nc.gpsimd.to_reg(v)
        return _nreg_cache[v]

    xT_in = nc.dram_tensor("xT", [F_IN, OWNP], BF16, kind="ExternalInput")
    idxE_in = nc.dram_tensor("idxE", [128, TOTSLOT // 16], dt.int16,
                             kind="ExternalInput")
    idxO_in = nc.dram_tensor("idxO", [128, TOTSLOT // 16], dt.int16,
                             kind="ExternalInput")
    wcat0_in = nc.dram_tensor("wcat0", [F_IN, C0], BF16, kind="ExternalInput")
    wr0_in = nc.dram_tensor("wr0", [F_IN, F_IN], BF16, kind="ExternalInput")
    wcat1_in = nc.dram_tensor("wcat1", [F_IN, C1], BF16, kind="ExternalInput")
    biasr1_in = nc.dram_tensor("bias_row1", [1, C1], BF16, kind="ExternalInput")
    bias0_in = nc.dram_tensor("bias0", [128, KCH], F32, kind="ExternalInput")
    out_own = nc.dram_tensor("out_own", [OWNP, OUT], F32, kind="ExternalOutput")

    tbl0_shard = nc.dram_tensor("tbl0_shard", [OWNP, ROW0], BF16)
    tbl1_shard = nc.dram_tensor("tbl1_shard", [OWNP, ROW1], BF16)
    reg0 = nc.dram_tensor("reg0", [VROWS, 2 * ROW0], BF16, addr_space="Shared")
    reg1 = nc.dram_tensor("reg1", [VROWS, 2 * ROW1], BF16, addr_space="Shared")
    rg = [list(range(C))]

    with tile.TileContext(nc) as tc, ExitStack() as ctx:
        const = ctx.enter_context(tc.tile_pool(name="const", bufs=1))
        wcat0_t = const.tile([128, KCH, C0], BF16)
        nc.sync.dma_start(wcat0_t[:], wcat0_in[:, :].rearrange(
            "(k p) c -> p k c", p=128))
        wr0_t = const.tile([128, KCH, F_IN], BF16)
        nc.sync.dma_start(wr0_t[:], wr0_in[:, :].rearrange(
            "(k p) c -> p k c", p=128))
        wcat1_t = const.tile([128, KCH, C1], BF16)
        nc.sync.dma_start(wcat1_t[:], wcat1_in[:, :].rearrange(
            "(k p) c -> p k c", p=128))
        biasr1_t = const.tile([1, C1], BF16)
        nc.sync.dma_start(biasr1_t[:], biasr1_in[:])
        bias0_t = const.tile([128, KCH], F32)
        nc.sync.dma_start(bias0_t[:], bias0_in[:])
        ones_t = const.tile([1, 128], BF16)
        nc.vector.memset(ones_t[:], 1.0)
        idxE_t = const.tile([128, TOTSLOT // 16], dt.int16)
        nc.sync.dma_start(idxE_t[:], idxE_in[:])
        idxO_t = const.tile([128, TOTSLOT // 16], dt.int16)
        nc.sync.dma_start(idxO_t[:], idxO_in[:])

        persist = ctx.enter_context(tc.tile_pool(name="persist", bufs=1))
        res0T = persist.tile([128, KCH, OWNP], BF16)
        h1T = persist.tile([128, KCH, OWNP], BF16)
        ad0 = persist.tile([128, NW, H], BF16)
        ad1 = persist.tile([128, NW, 1 + OUT], BF16)

        for _rep in range(repeat):
            # ---------------- setup: table0 rows, res0T, sentinels ----------
            with tc.tile_pool(name="xp", bufs=1) as xp, \
                 tc.tile_pool(name="sps", bufs=2, space="PSUM") as sps, \
                 tc.tile_pool(name="ssb", bufs=2) as ssb:
                xT_t = xp.tile([128, KCH, OWNP], BF16)
                nc.sync.dma_start(xT_t[:], xT_in[:, :].rearrange(
                    "(k p) n -> p k n", p=128))
                for w in range(NW):
                    ps = sps.tile([128, C0], F32, tag="t0")
                    for k in range(KCH):
                        nc.tensor.matmul(ps[:], xT_t[:, k, w * 128:(w + 1) * 128],
                                         wcat0_t[:, k, :],
                                         start=(k == 0), stop=(k == KCH - 1))
                    st = ssb.tile([128, C0], BF16, tag="st0")
                    nc.scalar.copy(st[:], ps[:])
                    nc.sync.dma_start(tbl0_shard[w * 128:(w + 1) * 128, :C0],
                                      st[:])
                # kick the collective off now so it overlaps the res0T
                # matmuls below (it only needs tbl0_shard)
                nc.sync.dma_start(
                    ad0[:], tbl0_shard[:, F_IN + H:F_IN + 2 * H].rearrange(
                        "(w p) e -> p w e", p=128))
                if NOCOLL:
                    nc.sync.dma_start(
                        reg0[2:2 + OWNP // 2, :],
                        tbl0_shard[:, :].rearrange("(v t) e -> v (t e)", t=2))
                else:
                    nc.gpsimd.collective_compute(
                        "AllGather", AL.bypass, replica_groups=rg,
                        ins=[tbl0_shard[:].opt()], outs=[reg0[2:, :].opt()])
                RT = 512
                for fc in range(KCH):
                    for n0 in range(0, OWNP, RT):
                        n1 = min(n0 + RT, OWNP)
                        ps = sps.tile([128, RT], F32, tag="r0")
                        for k in range(KCH):
                            nc.tensor.matmul(
                                ps[:, :n1 - n0],
                                wr0_t[:, k, fc * 128:(fc + 1) * 128],
                                xT_t[:, k, n0:n1],
                                start=(k == 0), stop=(k == KCH - 1))
                        nc.scalar.activation(res0T[:, fc, n0:n1],
                                             ps[:, :n1 - n0],
                                             ACT.Identity,
                                             bias=bias0_t[:, fc:fc + 1])
                # sentinels: view rows 0..1  (= table rows 0..3).
                # row 0 = zeros; row 1 starts with the PAD row (al = -1e30).
                sz = ssb.tile([1, 2 * ROW0], BF16, tag="sz")
                nc.vector.memset(sz[:], 0.0)
                sp = ssb.tile([1, 2 * ROW0], BF16, tag="sp")
                nc.vector.memset(sp[:], 0.0)
                nc.vector.memset(sp[:, F_IN:F_IN + H], -1e30)
                nc.sync.dma_start(reg0[0:1, :], sz[:])
                nc.sync.dma_start(reg0[1:2, :], sp[:])
                s1z = ssb.tile([1, 2 * ROW1], BF16, tag="s1z")
                nc.vector.memset(s1z[:], 0.0)
                s1p = ssb.tile([1, 2 * ROW1], BF16, tag="s1p")
                nc.vector.memset(s1p[:], 0.0)
                nc.vector.memset(s1p[:, OUT:OUT + 1], -1e30)
                nc.sync.dma_start(reg1[0:1, :], s1z[:])
                nc.sync.dma_start(reg1[1:2, :], s1p[:])

            # ---------------- layer 0 ----------------
            def gather_batch(gp, go, idx_off, slots, elem, regv_e, regv_o,
                             step):
                G = gp.tile([128, slots // 128, elem], BF16, tag="G")
                if NOGATHER:
                    nc.vector.memset(G[:], 0.0)
                    return G
                for c0 in range(0, slots, GCAP):
                    csl = min(GCAP, slots - c0)
                    Go = go.tile([128, GCAP // 128, elem], BF16, tag="Go")
                    nc.gpsimd.dma_gather(
                        out_ap=G[:, c0 // 128:(c0 + csl) // 128, :],
                        in_ap=regv_e,
                        idxs_ap=idxE_t[:, (idx_off + c0) // 16:
                                       (idx_off + c0 + csl) // 16],
                        num_idxs=csl, num_idxs_reg=nreg(csl), elem_size=elem,
                        elem_step=step)
                    nc.gpsimd.dma_gather(
                        out_ap=Go[:, :csl // 128, :],
                        in_ap=regv_o,
                        idxs_ap=idxO_t[:, (idx_off + c0) // 16:
                                       (idx_off + c0 + csl) // 16],
                        num_idxs=csl, num_idxs_reg=nreg(csl), elem_size=elem,
                        elem_step=step)
                    nc.vector.tensor_tensor(
                        G[:, c0 // 128:(c0 + csl) // 128, :],
                        G[:, c0 // 128:(c0 + csl) // 128, :],
                        Go[:, :csl // 128, :], AL.add)
                return G

            idx_off = 0
            with tc.tile_pool(name="g0", bufs=1) as gp, \
                 tc.tile_pool(name="go0", bufs=2) as go, \
                 tc.tile_pool(name="wk0", bufs=2) as wk:
                for (w0, nw_b, K) in batches:
                    slots = nw_b * K * 128
                    G = gather_batch(gp, go, idx_off, slots, ROW0,
                                     reg0[:, :ROW0], reg0[:, ROW0:],
                                     2 * ROW0)
                    idx_off += slots
                    Gv = G[:].rearrange("p (w k) e -> p w k e", w=nw_b)
                    s = wk.tile([128, nw_b, K, H], F32, tag="s")
                    nc.vector.tensor_tensor(
                        s[:], Gv[:, :, :, F_IN:F_IN + H],
                        ad0[:, w0:w0 + nw_b, :].unsqueeze(2)
                            .broadcast_to((128, nw_b, K, H)), AL.add)
                    nc.vector.scalar_tensor_tensor(s[:], s[:], NEG_SLOPE,
                                                   s[:], AL.mult, AL.max)
                    # per-lane max-shift: keeps exp args <= 0 (softmax
                    # shift-invariant; ACT exp is most accurate there)
                    mx = wk.tile([128, nw_b, H], F32, tag="mx")
                    nc.vector.reduce_max(
                        mx[:].unsqueeze(3),
                        s[:].rearrange("p w k h -> p w h k"),
                        axis=mybir.AxisListType.X)
                    nc.vector.tensor_tensor(
                        s[:], s[:],
                        mx[:].unsqueeze(2).broadcast_to((128, nw_b, K, H)),
                        AL.subtract)
                    Ex = wk.tile([128, nw_b, K, H], BF16, tag="E")
                    nc.scalar.activation(Ex[:], s[:], ACT.Exp)
                    nc.vector.tensor_tensor(
                        G[:, :, :F_IN].rearrange("p m (h d) -> p m h d", h=H),
                        G[:, :, :F_IN].rearrange("p m (h d) -> p m h d", h=H),
                        Ex[:].rearrange("p w k h -> p (w k) h").unsqueeze(3)
                            .broadcast_to((128, nw_b * K, H, D)), AL.mult)
                    nc.vector.tensor_copy(Gv[:, :, :, F_IN:F_IN + H], Ex[:])
                    U = wk.tile([128, nw_b, F_IN + H], F32, tag="U")
                    nc.vector.reduce_sum(
                        U[:].unsqueeze(3),
                        Gv[:, :, :, :F_IN + H].rearrange("p w k e -> p w e k"),
                        axis=mybir.AxisListType.X)
                    nc.vector.tensor_scalar(U[:, :, F_IN:], U[:, :, F_IN:],
                                            1e-16, None, AL.add)
                    rcp = wk.tile([128, nw_b, H], F32, tag="rcp")
                    nc.vector.reciprocal(rcp[:], U[:, :, F_IN:])
                    o0 = wk.tile([128, nw_b, F_IN], F32, tag="o0")
                    nc.vector.tensor_tensor(
                        o0[:].rearrange("p w (h d) -> p w h d", h=H),
                        U[:, :, :F_IN].rearrange("p w (h d) -> p w h d", h=H),
                        rcp[:].unsqueeze(3).broadcast_to((128, nw_b, H, D)),
                        AL.mult)
                    rT = wk.tile([128, nw_b, KCH, 128], BF16, tag="rT")
                    for wl in range(nw_b):
                        for k in range(KCH):
                            nc.sync.dma_start(
                                rT[:, wl, k, :],
                                res0T[:, k, (w0 + wl) * 128:(w0 + wl + 1) * 128],
                                transpose=True)
                    nc.vector.tensor_tensor(
                        o0[:], o0[:],
                        rT[:].rearrange("p w k e -> p w (k e)"), AL.add)
                    # ELU
                    mn = wk.tile([128, nw_b, F_IN], F32, tag="mn")
                    nc.vector.tensor_scalar(mn[:], o0[:], 0.0, None, AL.min)
                    nc.scalar.activation(mn[:], mn[:], ACT.Exp)
                    nc.vector.tensor_scalar(o0[:], o0[:], 0.0, None, AL.max)
                    nc.vector.tensor_tensor(o0[:], o0[:], mn[:], AL.add)
                    h1 = wk.tile([128, nw_b, F_IN], BF16, tag="h1")
                    nc.vector.tensor_scalar(h1[:], o0[:], 1.0, None,
                                            AL.subtract)
                    for wl in range(nw_b):
                        for k in range(KCH):
                            nc.sync.dma_start(
                                h1T[:, k, (w0 + wl) * 128:(w0 + wl + 1) * 128],
                                h1[:, wl, k * 128:(k + 1) * 128],
                                transpose=True)

            # ---------------- table1 rows ----------------
            with tc.tile_pool(name="t1ps", bufs=2, space="PSUM") as tps, \
                 tc.tile_pool(name="t1sb", bufs=2) as tsb:
                for w in range(NW):
                    ps = tps.tile([128, C1], F32, tag="t1")
                    for k in range(KCH):
                        nc.tensor.matmul(ps[:], h1T[:, k, w * 128:(w + 1) * 128],
                                         wcat1_t[:, k, :], start=(k == 0),
                                         stop=(k == KCH - 1 and not BIAS1))
                    if BIAS1:
                        nc.tensor.matmul(ps[:], ones_t[:], biasr1_t[:],
                                         start=False, stop=True)
                    st = tsb.tile([128, C1], BF16, tag="st1")
                    nc.scalar.copy(st[:], ps[:])
                    nc.sync.dma_start(tbl1_shard[w * 128:(w + 1) * 128, :C1],
                                      st[:])
            nc.sync.dma_start(
                ad1[:], tbl1_shard[:, OUT + 1:2 * OUT + 2].rearrange(
                    "(w p) e -> p w e", p=128))

            if NOCOLL:
                nc.sync.dma_start(
                    reg1[2:2 + OWNP // 2, :],
                    tbl1_shard[:, :].rearrange("(v t) e -> v (t e)", t=2))
            else:
                nc.gpsimd.collective_compute(
                    "AllGather", AL.bypass, replica_groups=rg,
                    ins=[tbl1_shard[:].opt()], outs=[reg1[2:, :].opt()])

            # ---------------- layer 1 ----------------
            GE1 = 128   # gather elems (covers hW1|als)
            idx_off = 0
            with tc.tile_pool(name="g1", bufs=1) as gp, \
                 tc.tile_pool(name="go1", bufs=2) as go, \
                 tc.tile_pool(name="wk1", bufs=2) as wk:
                for (w0, nw_b, K) in batches:
                    slots = nw_b * K * 128
                    G = gather_batch(gp, go, idx_off, slots, GE1,
                                     reg1[:, :GE1], reg1[:, ROW1:ROW1 + GE1],
                                     2 * ROW1)
                    idx_off += slots
                    Gv = G[:].rearrange("p (w k) e -> p w k e", w=nw_b)
                    s = wk.tile([128, nw_b, K, 1], F32, tag="s")
                    nc.vector.tensor_tensor(
                        s[:], Gv[:, :, :, OUT:OUT + 1],
                        ad1[:, w0:w0 + nw_b, 0:1].unsqueeze(2)
                            .broadcast_to((128, nw_b, K, 1)), AL.add)
                    nc.vector.scalar_tensor_tensor(s[:], s[:], NEG_SLOPE,
                                                   s[:], AL.mult, AL.max)
                    mx = wk.tile([128, nw_b, 1], F32, tag="mx")
                    nc.vector.reduce_max(
                        mx[:].unsqueeze(3),
                        s[:].rearrange("p w k h -> p w h k"),
                        axis=mybir.AxisListType.X)
                    nc.vector.tensor_tensor(
                        s[:], s[:],
                        mx[:].unsqueeze(2).broadcast_to((128, nw_b, K, 1)),
                        AL.subtract)
                    Ex = wk.tile([128, nw_b, K, 1], BF16, tag="E")
                    nc.scalar.activation(Ex[:], s[:], ACT.Exp)
                    nc.vector.tensor_tensor(
                        G[:, :, :OUT], G[:, :, :OUT],
                        Ex[:].rearrange("p w k h -> p (w k) h")
                            .broadcast_to((128, nw_b * K, OUT)), AL.mult)
                    nc.vector.tensor_copy(Gv[:, :, :, OUT:OUT + 1], Ex[:])
                    U = wk.tile([128, nw_b, OUT + 1], F32, tag="U")
                    nc.vector.reduce_sum(
                        U[:].unsqueeze(3),
                        Gv[:, :, :, :OUT + 1].rearrange("p w k e -> p w e k"),
                        axis=mybir.AxisListType.X)
                    nc.vector.tensor_scalar(U[:, :, OUT:], U[:, :, OUT:],
                                            1e-16, None, AL.add)
                    rcp = wk.tile([128, nw_b, 1], F32, tag="rcp")
                    nc.vector.reciprocal(rcp[:], U[:, :, OUT:])
                    o = wk.tile([128, nw_b, OUT], F32, tag="o")
                    nc.vector.tensor_tensor(
                        o[:], U[:, :, :OUT],
                        rcp[:].broadcast_to((128, nw_b, OUT)), AL.mult)
                    nc.vector.tensor_tensor(o[:], o[:],
                                            ad1[:, w0:w0 + nw_b, 1:],
                                            AL.add)
                    # log_softmax with max-shift
                    mxo = wk.tile([128, nw_b, 1], F32, tag="mxo")
                    nc.vector.reduce_max(mxo[:], o[:], axis=mybir.AxisListType.X)
                    nc.vector.tensor_tensor(
                        o[:], o[:], mxo[:].broadcast_to((128, nw_b, OUT)),
                        AL.subtract)
                    ev = wk.tile([128, nw_b, OUT], F32, tag="ev")
                    nc.scalar.activation(ev[:], o[:], ACT.Exp)
                    sv = wk.tile([128, nw_b, 1], F32, tag="sv")
                    nc.vector.reduce_sum(sv[:], ev[:], axis=mybir.AxisListType.X)
                    nc.scalar.activation(sv[:], sv[:], ACT.Ln)
                    nc.vector.tensor_tensor(
                        o[:], o[:], sv[:].broadcast_to((128, nw_b, OUT)),
                        AL.subtract)
                    nc.sync.dma_start(
                        out_own[w0 * 128:(w0 + nw_b) * 128, :].rearrange(
                            "(w p) e -> p w e", p=128), o[:])

    nc.compile()
    return nc


# ----------------------------------------------------------------- entrypoint

def kernel(x, edge_index, W0, a_src0, a_dst0, b0, Wr0, br0,
           W1, a_src1, a_dst1, b1, Wr1, br1):
    """Full-input GAT kernel: shards across 8 NeuronCores internally."""
    x = np.asarray(x)
    edge_index = np.asarray(edge_index)
    N, F_in = x.shape
    E = edge_index.shape[1]
    H, D = np.asarray(a_src0).shape
    OUT = np.asarray(a_src1).shape[1]
    cfg = make_cfg(N, E, F_in, H, D, OUT, _CORES)
    weights = dict(
        W0=np.asarray(W0, np.float32), a_src0=np.asarray(a_src0, np.float32),
        a_dst0=np.asarray(a_dst0, np.float32), b0=np.asarray(b0, np.float32),
        Wr0=np.asarray(Wr0, np.float32), br0=np.asarray(br0, np.float32),
        W1=np.asarray(W1, np.float32), a_src1=np.asarray(a_src1, np.float32),
        a_dst1=np.asarray(a_dst1, np.float32), b1=np.asarray(b1, np.float32),
        Wr1=np.asarray(Wr1, np.float32), br1=np.asarray(br1, np.float32))
    in_maps, meta = prepare(x.astype(np.float32), edge_index, weights, cfg)
    nc = build(cfg, meta)
    from concourse.bass_utils import run_bass_kernel_spmd
    res = run_bass_kernel_spmd(nc, in_maps, list(range(_CORES)))
    core_of, pos_of = meta["core_of"], meta["pos_of"]
    per_core = [np.asarray(res.results[c]["out_own"], np.float32)
                for c in range(_CORES)]
    stacked = np.stack(per_core)                       # [C, OWNP, OUT]
    out = stacked[core_of, pos_of]                     # [N, OUT]
    return out
